# revision 1
# baseline (speedup 1.0000x reference)
"""Trainium2 Bass kernel for a causal self-attention block (GQA + per-head
RMS-norm + RoPE + learned q-gain), sharded over 8 NeuronCores.

Sharding: data-parallel over batch (B=2) as the outer axis x tensor-parallel
over head groups (4 groups of 4 query heads, each owning one KV head).
core = b*4 + g. Each core computes the full attention for its 4 heads and a
*partial* output projection (its 256 in-dims of Wproj); the host sums the 4
partials per batch element and transposes back.

Inside a core everything is computed in a transposed ("T") layout for the
attention matmuls: scores are built as S^T[k, q] = K @ Q^T so the PV matmul
can contract over keys on the partition axis; a row of ones appended to V
yields the softmax denominator for free.
"""

import math

import numpy as np

import concourse.bacc as bacc
import concourse.bass as bass
import concourse.tile as tile
from concourse import mybir
from concourse.bass import ts
from concourse.bass_utils import run_bass_kernel_spmd
from concourse.masks import make_identity

# Problem dims (hardcoded per contract).
B, S, D, H, KV, HD = 2, 2048, 1024, 16, 4, 64
NH = H // KV          # 4 query heads per core (one KV group)
GD = NH * HD          # 256 out-dims of Wq per group
P = 128               # partitions
NST = S // P          # 16 sequence tiles
JW = 512              # query-block width for attention
NJ = S // JW          # 4 query blocks
NC = 8                # cores
ROPE_BASE = 10000.0
RMS_EPS = 1.1920929e-07
F32 = mybir.dt.float32
F32R = mybir.dt.float32r
AXX = mybir.AxisListType.X
ACT = mybir.ActivationFunctionType


def _build_program(reps=1):
    # Bacc (vs raw Bass) runs the TRN2 lowering passes: matmul waits moved to
    # ldweights, sync-wait splitting, act-table/library load insertion.
    # reps>1 replicates the whole body for slope-based benchmarking.
    nc = bacc.Bacc("TRN2", target_bir_lowering=False, debug=False)

    # f32r tensors hold ordinary f32 bits; the declaration lets the PE run
    # its full-rate fp32 path (4x faster than strict fp32 matmul).
    xT = nc.dram_tensor("xT", [D, S], F32R, kind="ExternalInput").ap()
    wqkv = nc.dram_tensor("wqkv", [D, GD + 2 * HD], F32R, kind="ExternalInput").ap()
    wp2 = nc.dram_tensor("wp2", [P, 2 * D], F32R, kind="ExternalInput").ap()
    cosn = nc.dram_tensor("cosn", [P, NST * HD], F32, kind="ExternalInput").ap()
    sinn = nc.dram_tensor("sinn", [P, NST * 32], F32, kind="ExternalInput").ap()
    masks = nc.dram_tensor("masks", [P, 4 * JW], F32, kind="ExternalInput").ap()
    qg8 = nc.dram_tensor("qg8", [1, NH], F32, kind="ExternalInput").ap()
    ypt = nc.dram_tensor("ypt", [D, S], F32, kind="ExternalOutput").ap()

    with tile.TileContext(nc) as tc:
        for _ in range(reps):
            _body(tc, xT, wqkv, wp2, cosn, sinn, masks, qg8, ypt)
    nc.compile()
    return nc


def _body(tc, xT, wqkv, wp2, cosn, sinn, masks, qg8, ypt):
    nc = tc.nc
    NQKV = GD + 2 * HD  # 384

    with tc.tile_pool(name="consts", bufs=1) as consts:
        # Persistent SBUF state.
        wp_sb = consts.tile([P, 2, D], F32R, name="wp_sb")
        cos_sb = consts.tile([P, NST, HD], F32, name="cos_sb")
        sin_sb = consts.tile([P, NST, 32], F32, name="sin_sb")
        mask_sb = consts.tile([P, 4, JW], F32, name="mask_sb")
        qg8_sb = consts.tile([P, NH], F32, name="qg8_sb")
        ident = consts.tile([P, P], F32, name="ident")
        # qT/kT zero-padded to 128 partitions so attention matmuls run K=128
        # (no PE tiling-mode switches); rows 64-127 stay zero.
        qT_sb = consts.tile([P, NH, S], F32R, name="qT_sb")
        kT_sb = consts.tile([P, S], F32R, name="kT_sb")
        # V with a ones-column (65th) so PV accumulates softmax denominators.
        v_sb = consts.tile([P, NST, HD + 1], F32R, name="v_sb")
        # Normalized y^T, head pairs stacked on partitions for the out-proj.
        y_sb = consts.tile([P, 2, S], F32R, name="y_sb")
        # Selector matrix (row 64 all-ones) broadcasts the softmax denominator
        # over partitions via a plain K=128 matmul (no PE mode switch).
        sel64 = consts.tile([P, P], F32R, name="sel64")
        # Rotating staging rows for the reciprocal denominators: rows 0-63 and
        # 65-127 stay zero forever; row 64 is rewritten per use.
        bc0 = consts.tile([P, JW], F32R, name="bc0")
        bc1 = consts.tile([P, JW], F32R, name="bc1")

        nc.sync.dma_start(out=wp_sb, in_=wp2.rearrange("p (c m) -> p c m", c=2))
        nc.sync.dma_start(out=cos_sb, in_=cosn.rearrange("p (t f) -> p t f", f=HD))
        nc.sync.dma_start(out=sin_sb, in_=sinn.rearrange("p (t f) -> p t f", f=32))
        nc.sync.dma_start(out=mask_sb, in_=masks.rearrange("p (m c) -> p m c", c=JW))
        nc.gpsimd.dma_start(out=qg8_sb, in_=qg8.to_broadcast([P, NH]))
        make_identity(nc, ident)
        # f32r tiles can't be memset directly (ISA); fill via f32 -> f32r
        # broadcast copies, which are legal rounding producers.
        z1 = consts.tile([P, 1], F32, name="z1")
        o1 = consts.tile([P, 1], F32, name="o1")
        nc.vector.memset(z1, 0.0)
        nc.vector.memset(o1, 1.0)
        nc.vector.tensor_copy(
            v_sb[:, :, HD : HD + 1], o1[:, None, :].broadcast_to([P, NST, 1])
        )
        nc.vector.tensor_copy(
            qT_sb[HD:P, :, :], z1[HD:P, :][:, None, :].broadcast_to([HD, NH, S])
        )
        nc.vector.tensor_copy(kT_sb[HD:P, :], z1[HD:P, :].broadcast_to([HD, S]))
        nc.vector.tensor_copy(sel64, z1.broadcast_to([P, P]))
        nc.vector.tensor_copy(
            sel64[HD : HD + 1, :], o1[HD : HD + 1, :].broadcast_to([1, P])
        )
        nc.vector.tensor_copy(bc0, z1.broadcast_to([P, JW]))
        nc.vector.tensor_copy(bc1, z1.broadcast_to([P, JW]))

        # ---------------- Phase 1: QKV proj + RMS + RoPE + transposes -----
        with tc.tile_pool(name="ph1c", bufs=1) as ph1c:
            xT_sb = ph1c.tile([P, 8, S], F32R, name="xT_sb")
            w_sb = ph1c.tile([P, 8, NQKV], F32R, name="w_sb")
            xTr = xT.rearrange("(c p) s -> p c s", p=P)
            for c in range(8):
                nc.sync.dma_start(out=xT_sb[:, c, :], in_=xTr[:, c, :])
            nc.sync.dma_start(out=w_sb, in_=wqkv.rearrange("(c p) n -> p c n", p=P))

            with (
                tc.tile_pool(name="p1w", bufs=3) as work,
                tc.tile_pool(name="p1ps", bufs=3, space="PSUM") as psP,
                tc.tile_pool(name="p1pt", bufs=4, space="PSUM") as psT,
            ):
                for i in range(NST):
                    qkv_ps = psP.tile([P, NQKV], F32, name=f"qkv_ps{i}", tag="qkv")
                    for c in range(8):
                        nc.tensor.matmul(
                            qkv_ps,
                            lhsT=xT_sb[:, c, ts(i, P)],
                            rhs=w_sb[:, c, :],
                            start=(c == 0),
                            stop=(c == 7),
                        )
                    # V tile straight out of PSUM.
                    nc.scalar.copy(v_sb[:, i, 0:HD], qkv_ps[:, GD + HD : NQKV])

                    # Merged RMS stats for 4 q heads + k (5 slabs of 64).
                    sq5 = work.tile([P, 5 * HD], F32, name=f"sq5_{i}", tag="sq5")
                    nc.scalar.square(sq5, qkv_ps[:, 0 : 5 * HD])
                    ss5 = work.tile([P, 5], F32, name=f"ss5_{i}", tag="ss5")
                    nc.vector.reduce_sum(
                        ss5, sq5.rearrange("p (h d) -> p h d", d=HD), axis=AXX
                    )
                    m5 = work.tile([P, 5], F32, name=f"m5_{i}", tag="m5")
                    nc.vector.tensor_scalar(
                        out=m5, in0=ss5, scalar1=1.0 / HD, scalar2=RMS_EPS,
                        op0=mybir.AluOpType.mult, op1=mybir.AluOpType.add,
                    )
                    s5 = work.tile([P, 5], F32, name=f"s5_{i}", tag="s5")
                    nc.scalar.activation(s5, m5, ACT.Sqrt)
                    r5 = work.tile([P, 5], F32, name=f"r5_{i}", tag="r5")
                    nc.vector.reciprocal(r5, s5)
                    # One Newton step on rsqrt corrects sqrt-table + recip err.
                    t5 = work.tile([P, 5], F32, name=f"t5_{i}", tag="t5")
                    nc.vector.tensor_mul(t5, m5, r5)
                    nc.vector.tensor_mul(t5, t5, r5)
                    nc.vector.tensor_scalar(
                        out=t5, in0=t5, scalar1=-0.5, scalar2=1.5,
                        op0=mybir.AluOpType.mult, op1=mybir.AluOpType.add,
                    )
                    nc.vector.tensor_mul(r5, r5, t5)
                    # Fold gain/8 into the q scales (k slab untouched).
                    nc.vector.tensor_mul(r5[:, 0:NH], r5[:, 0:NH], qg8_sb)

                    # Scale + RoPE, q and k batched (cos table duplicated to
                    # 64 wide; rot = qks*cosd then +/- the swapped-half * sin).
                    q5 = qkv_ps[:, 0 : 5 * HD].rearrange("p (h d) -> p h d", d=HD)
                    qks = work.tile([P, 5, HD], F32, name=f"qks_{i}", tag="qks")
                    nc.vector.tensor_mul(
                        qks, q5, r5[:, :, None].broadcast_to([P, 5, HD])
                    )
                    rot = work.tile([P, 5, HD], F32, name=f"rot_{i}", tag="rot")
                    cb = cos_sb[:, i, :][:, None, :].broadcast_to([P, 5, HD])
                    sb_ = sin_sb[:, i, :][:, None, :].broadcast_to([P, 5, 32])
                    nc.vector.tensor_mul(rot, qks, cb)
                    m2a = work.tile([P, 5, 32], F32, name=f"m2a_{i}", tag="m2a")
                    nc.vector.tensor_mul(m2a, qks[:, :, 32:HD], sb_)
                    m2b = work.tile([P, 5, 32], F32, name=f"m2b_{i}", tag="m2b")
                    nc.vector.tensor_mul(m2b, qks[:, :, 0:32], sb_)
                    nc.vector.tensor_add(rot[:, :, 0:32], rot[:, :, 0:32], m2a)
                    nc.vector.tensor_sub(rot[:, :, 32:HD], rot[:, :, 32:HD], m2b)

                    # Transpose each slab to [d, s] layout.
                    for slab in range(5):
                        trq = psT.tile([HD, P], F32, name=f"tr{i}_{slab}", tag="tr")
                        nc.tensor.transpose(trq, rot[:, slab, :], ident)
                        if slab < NH:
                            nc.vector.tensor_copy(qT_sb[0:HD, slab, ts(i, P)], trq)
                        else:
                            nc.vector.tensor_copy(kT_sb[0:HD, ts(i, P)], trq)

        # ---------------- Phase 2: attention --------------------------------
        with (
            tc.tile_pool(name="p2w", bufs=3) as workp,
            tc.tile_pool(name="p2s", bufs=2, space="PSUM") as psS,
            tc.tile_pool(name="p2y", bufs=2, space="PSUM") as psY,
            tc.tile_pool(name="p2b", bufs=2, space="PSUM") as psB,
        ):
            for h in range(NH):
                for j in range(NJ):
                    nt = 4 * (j + 1)  # valid k-tiles for this q block
                    y_ps = psY.tile([HD + 1, JW], F32, name=f"y_ps{h}_{j}", tag="y")
                    qh = qT_sb[:, h, ts(j, JW)]
                    for cc in range(nt // 2):
                        st = psS.tile([P, 2 * JW], F32, name=f"st{h}_{j}_{cc}", tag="st")
                        for u in range(2):
                            t = 2 * cc + u
                            nc.tensor.matmul(
                                st[:, ts(u, JW)],
                                lhsT=kT_sb[:, ts(t, P)],
                                rhs=qh,
                                start=True,
                                stop=True,
                            )
                        p_sb = workp.tile([P, 2 * JW], F32R, name=f"p{h}_{j}_{cc}", tag="p")
                        nc.scalar.activation(p_sb, st, ACT.Exp)
                        for u in range(2):
                            m = 2 * cc + u - 4 * j
                            if m >= 0:  # diagonal tile: zero the future keys
                                nc.vector.tensor_mul(
                                    p_sb[:, ts(u, JW)], p_sb[:, ts(u, JW)],
                                    mask_sb[:, m, :],
                                )
                        for u in range(2):
                            t = 2 * cc + u
                            nc.tensor.matmul(
                                y_ps,
                                lhsT=v_sb[:, t, :],
                                rhs=p_sb[:, ts(u, JW)],
                                start=(t == 0),
                                stop=(t == nt - 1),
                            )
                    # Softmax normalization: row HD of y_ps is the denom.
                    bc = (bc0, bc1)[(h * NJ + j) % 2]
                    lrow = workp.tile([HD + 1, JW], F32, name=f"lr{h}_{j}", tag="lrow")
                    nc.vector.reciprocal(lrow[HD : HD + 1, :], y_ps[HD : HD + 1, :])
                    nc.vector.tensor_copy(bc[HD : HD + 1, :], lrow[HD : HD + 1, :])
                    bcp = psB.tile([P, JW], F32, name=f"bcp{h}_{j}", tag="bcp")
                    nc.tensor.matmul(
                        bcp, lhsT=sel64, rhs=bc, start=True, stop=True
                    )
                    bcs = workp.tile([HD, JW], F32, name=f"bcs{h}_{j}", tag="bcs")
                    nc.vector.tensor_copy(bcs, bcp[0:HD, :])
                    if h % 2 == 0:
                        nc.vector.tensor_mul(
                            y_sb[0:HD, h // 2, ts(j, JW)], y_ps[0:HD, :], bcs
                        )
                    else:
                        ytmp = workp.tile([HD, JW], F32R, name=f"yt{h}_{j}", tag="ytmp")
                        nc.vector.tensor_mul(ytmp, y_ps[0:HD, :], bcs)
                        nc.sync.dma_start(
                            out=y_sb[HD:P, h // 2, ts(j, JW)], in_=ytmp
                        )

        # ---------------- Phase 3: output projection (partial) --------------
        with (
            tc.tile_pool(name="p3w", bufs=4) as worko,
            tc.tile_pool(name="p3ps", bufs=4, space="PSUM") as psO,
        ):
            for m in range(D // P):
                for j in range(NJ):
                    op_ps = psO.tile([P, JW], F32, name=f"op{m}_{j}", tag="op")
                    for c in range(2):
                        nc.tensor.matmul(
                            op_ps,
                            lhsT=wp_sb[:, c, ts(m, P)],
                            rhs=y_sb[:, c, ts(j, JW)],
                            start=(c == 0),
                            stop=(c == 1),
                        )
                    o_sb = worko.tile([P, JW], F32, name=f"o{m}_{j}", tag="o")
                    eng = nc.vector if (m + j) % 2 == 0 else nc.scalar
                    if eng is nc.vector:
                        nc.vector.tensor_copy(o_sb, op_ps)
                    else:
                        nc.scalar.copy(o_sb, op_ps)
                    nc.sync.dma_start(out=ypt[ts(m, P), ts(j, JW)], in_=o_sb)


_PROG = None


def _get_program():
    global _PROG
    if _PROG is None:
        _PROG = _build_program()
    return _PROG


def _host_tables():
    inv_freq = (1.0 / (ROPE_BASE ** (np.arange(0, HD, 2, dtype=np.float32) / HD))).astype(
        np.float32
    )
    t = np.arange(S, dtype=np.float32)
    freqs = t[:, None] * inv_freq[None, :]  # [S, 32]
    cosf = np.cos(freqs).astype(np.float32)
    sinf = np.sin(freqs).astype(np.float32)
    # natural per-s-tile layout: [p, tile, freq]
    cosd = np.concatenate([cosf, cosf], axis=1)  # [S, 64]
    cosn = np.ascontiguousarray(
        cosd.reshape(NST, P, HD).transpose(1, 0, 2).reshape(P, NST * HD)
    )
    sinn = np.ascontiguousarray(
        sinf.reshape(NST, P, 32).transpose(1, 0, 2).reshape(P, NST * 32)
    )
    p_idx = np.arange(P)[:, None]
    c_idx = np.arange(JW)[None, :]
    mlist = [(c_idx >= m * P + p_idx).astype(np.float32) for m in range(4)]
    masks = np.ascontiguousarray(np.concatenate(mlist, axis=1))  # [128, 2048]
    return cosn, sinn, masks


def _in_maps(x, Wq, Wk, Wv, Wproj, q_gain):
    cosn, sinn, masks = _host_tables()
    maps = []
    for core in range(NC):
        b, g = divmod(core, KV)
        xT = np.ascontiguousarray(x[b].T)  # [D, S]
        wqkv = np.ascontiguousarray(
            np.concatenate(
                [
                    Wq[g * GD : (g + 1) * GD].T,
                    Wk[g * HD : (g + 1) * HD].T,
                    Wv[g * HD : (g + 1) * HD].T,
                ],
                axis=1,
            )
        )  # [D, 384]
        wsl = Wproj[:, g * GD : (g + 1) * GD].T.reshape(NH, HD, D)  # [head, d, m]
        wp2 = np.ascontiguousarray(
            np.stack(
                [
                    np.concatenate([wsl[0], wsl[1]], axis=0),
                    np.concatenate([wsl[2], wsl[3]], axis=0),
                ],
                axis=1,
            ).reshape(P, 2 * D)
        )
        qg8 = np.ascontiguousarray(
            (q_gain[g * NH : (g + 1) * NH] / 8.0).astype(np.float32).reshape(1, NH)
        )
        maps.append(
            {
                "xT": xT,
                "wqkv": wqkv,
                "wp2": wp2,
                "cosn": cosn,
                "sinn": sinn,
                "masks": masks,
                "qg8": qg8,
            }
        )
    return maps


def kernel(x, Wq, Wk, Wv, Wproj, q_gain, _collect=None):
    x = np.asarray(x, dtype=np.float32)
    Wq = np.asarray(Wq, dtype=np.float32)
    Wk = np.asarray(Wk, dtype=np.float32)
    Wv = np.asarray(Wv, dtype=np.float32)
    Wproj = np.asarray(Wproj, dtype=np.float32)
    q_gain = np.asarray(q_gain, dtype=np.float32)

    nc = _get_program()
    maps = _in_maps(x, Wq, Wk, Wv, Wproj, q_gain)
    res = run_bass_kernel_spmd(nc, maps, core_ids=list(range(NC)))
    if _collect is not None:
        _collect.append(res)

    out = np.zeros((B, S, D), dtype=np.float64)
    for core in range(NC):
        b, _ = divmod(core, KV)
        out[b] += res.results[core]["ypt"].T.astype(np.float64)
    return out.astype(np.float32)



# revision 8
# speedup vs baseline: 1.2424x; 1.2424x over previous
"""Trainium2 Bass kernel for a causal self-attention block (GQA + per-head
RMS-norm + RoPE + learned q-gain), sharded over 8 NeuronCores.

Sharding: data-parallel over batch (B=2) x tensor-parallel over KV groups
(4 groups of 4 query heads). core = b*4 + g. Each core computes attention for
its 4 heads and a partial output projection (its 256 in-dims of Wproj); the
host sums the 4 partials per batch element.

This version is a software-pipelined rewrite tuned for engine balance:
  - one fused instruction stream: QKV chunks (U), attention blocks (B) and
    output-projection blocks (C) interleave so the PE never idles long enough
    to drop out of its warm clock state.
  - q is stored in head-PAIR layout (partitions 0-63 = even head dims,
    64-127 = odd head dims, straight out of a single 128x128 PE transpose);
    two zero-padded copies of k^T (kTe: k in rows 0-63, kTo: rows 64-127)
    let each head's score matmul contract only its half.
  - attention works on the causal band only: score/exp/mask/PV widths shrink
    on diagonal tiles, with a single shared triangular mask.
  - RMS-norm rsqrt is computed with a bitcast magic-constant seed + Newton
    steps on the DVE, so the scalar engine only ever loads the exp table set.
  - p/v/y/Wproj run in bf16 (fp32 PSUM accumulation); q/k scores stay fp32r.
  - softmax denominator rides row 0 of the PV matmul via a ones-column in V;
    reciprocal_approx_fast + gpsimd partition_broadcast normalize it.
"""

import math

import numpy as np

import concourse.bacc as bacc
import concourse.bass as bass
import concourse.tile as tile
from concourse import mybir
from concourse.bass import ts
from concourse.bass_utils import run_bass_kernel_spmd
from concourse.masks import make_identity

# Problem dims (hardcoded per contract).
B, S, D, H, KV, HD = 2, 2048, 1024, 16, 4, 64
NH = H // KV          # 4 query heads per core (one KV group)
GD = NH * HD          # 256 out-dims of Wq per group
P = 128               # partitions
NST = S // P          # 16 sequence tiles
JW = 512              # query-block width for attention
NJ = S // JW          # 4 query blocks
NC = 8                # cores
ROPE_BASE = 10000.0
RMS_EPS = 1.1920929e-07
F32 = mybir.dt.float32
F32R = mybir.dt.float32r
BF16 = mybir.dt.bfloat16
I32 = mybir.dt.int32
AXX = mybir.AxisListType.X
ACT = mybir.ActivationFunctionType
ALU = mybir.AluOpType
NQKV = GD + 2 * HD    # 384


def _build_program(reps=1):
    nc = bacc.Bacc("TRN2", target_bir_lowering=False, debug=False)

    xT = nc.dram_tensor("xT", [D, S], F32R, kind="ExternalInput").ap()
    wqkv = nc.dram_tensor("wqkv", [D, NQKV], F32R, kind="ExternalInput").ap()
    wp2 = nc.dram_tensor("wp2", [P, 2 * D], BF16, kind="ExternalInput").ap()
    cos5 = nc.dram_tensor("cos5", [P, NST * 5 * HD], F32, kind="ExternalInput").ap()
    sin5 = nc.dram_tensor("sin5", [P, NST * 5 * 32], F32, kind="ExternalInput").ap()
    tri = nc.dram_tensor("tri", [P, JW], BF16, kind="ExternalInput").ap()
    qg8 = nc.dram_tensor("qg8", [1, NH], F32, kind="ExternalInput").ap()
    ypt = nc.dram_tensor("ypt", [D, S], BF16, kind="ExternalOutput").ap()

    with tile.TileContext(nc) as tc:
        for _ in range(reps):
            _body(tc, xT, wqkv, wp2, cos5, sin5, tri, qg8, ypt)
    nc.compile()
    return nc


def _body(tc, xT, wqkv, wp2, cos5, sin5, tri, qg8, ypt):
    nc = tc.nc
    xTr = xT.rearrange("(c p) s -> p c s", p=P)

    with (
        tc.tile_pool(name="consts", bufs=1) as consts,
        tc.tile_pool(name="xtp", bufs=2) as xtp,
        tc.tile_pool(name="wk", bufs=3) as wk,
        tc.tile_pool(name="rwk", bufs=2) as rwk,
        tc.tile_pool(name="pwk", bufs=3) as pwk,
        tc.tile_pool(name="nwk", bufs=2) as nwk,
        tc.tile_pool(name="psmisc", bufs=2, space="PSUM") as psmisc,
        tc.tile_pool(name="psst", bufs=2, space="PSUM") as psst,
        tc.tile_pool(name="psy", bufs=2, space="PSUM") as psy,
    ):
        # ---------------- persistent SBUF state ----------------
        w_sb = consts.tile([P, 8, NQKV], F32R, name="w_sb")
        wp_sb = consts.tile([P, 2, D], BF16, name="wp_sb")
        cos_sb = consts.tile([P, NST, 5 * HD], F32, name="cos_sb")
        sin_sb = consts.tile([P, NST, 5 * 32], F32, name="sin_sb")
        tri_sb = consts.tile([P, JW], BF16, name="tri_sb")
        qg8_sb = consts.tile([P, NH], F32, name="qg8_sb")
        ident = consts.tile([P, P], F32, name="ident")
        qT2 = consts.tile([P, 2, S], F32R, name="qT2")
        kTe = consts.tile([P, S], F32R, name="kTe")
        kTo = consts.tile([P, S], F32R, name="kTo")
        # PV stationary operand: col 0 = ones (softmax denominator -> PSUM row
        # 0, where the custom recip/broadcast ops are legal), cols 1-63 = zero,
        # cols 64-127 = v dims (y lands at rows 64-127, 32-aligned).
        v_sb = consts.tile([P, NST, P], BF16, name="v_sb")
        y_sb = consts.tile([P, 2, S], BF16, name="y_sb")
        qkv_sb = consts.tile([P, NST, 5 * HD], F32, name="qkv_sb")
        ss_all = consts.tile([P, NST * 5], F32, name="ss_all")
        r_all = consts.tile([P, NST * 5], F32, name="r_all")

        nc.sync.dma_start(out=w_sb, in_=wqkv.rearrange("(c p) n -> p c n", p=P))
        nc.sync.dma_start(out=wp_sb, in_=wp2.rearrange("p (c m) -> p c m", c=2))
        nc.sync.dma_start(out=cos_sb, in_=cos5.rearrange("p (t f) -> p t f", t=NST))
        nc.sync.dma_start(out=sin_sb, in_=sin5.rearrange("p (t f) -> p t f", t=NST))
        nc.sync.dma_start(out=tri_sb, in_=tri)
        nc.gpsimd.dma_start(out=qg8_sb, in_=qg8.to_broadcast([P, NH]))
        make_identity(nc, ident)

        z1 = consts.tile([P, 1], F32, name="z1")
        nc.vector.memset(z1, 0.0)
        ob = consts.tile([P, 1], BF16, name="ob")
        nc.gpsimd.memset(ob, 1.0)
        zb = consts.tile([P, 1], BF16, name="zb")
        nc.gpsimd.memset(zb, 0.0)
        # ones-column (index 0) of V => softmax denominator at PSUM row 0
        nc.vector.tensor_copy(
            v_sb[:, :, 0:1], ob[:, None, :].broadcast_to([P, NST, 1])
        )
        nc.vector.tensor_copy(
            v_sb[:, :, 1:HD], zb[:, None, :].broadcast_to([P, NST, HD - 1])
        )
        # zero halves of the two padded k^T copies
        nc.vector.tensor_copy(kTe[HD:P, :], z1[HD:P, :].broadcast_to([HD, S]))
        nc.gpsimd.tensor_copy(kTo[0:HD, :], z1[0:HD, :].broadcast_to([HD, S]))

        # ---------------- pipelined stream ----------------
        def u_chunk(jb):
            """QKV proj + RMS + RoPE + transposes for s-tiles 4jb..4jb+3."""
            xt = xtp.tile([P, 8, JW], F32R, name=f"xt{jb}", tag="xt")
            for c in range(8):
                nc.sync.dma_start(out=xt[:, c, :], in_=xTr[:, c, ts(jb, JW)])
            for il in range(4):
                i = 4 * jb + il
                qkv_ps = psmisc.tile([P, NQKV], F32, name=f"qkv{i}", tag="mi")
                for c in range(8):
                    nc.tensor.matmul(
                        qkv_ps,
                        lhsT=xt[:, c, ts(il, P)],
                        rhs=w_sb[:, c, :],
                        start=(c == 0),
                        stop=(c == 7),
                    )
                # stage q,k (f32) and v (bf16); square+reduce for RMS stats
                nc.vector.tensor_copy(qkv_sb[:, i, :], qkv_ps[:, 0 : 5 * HD])
                nc.vector.tensor_copy(v_sb[:, i, HD:P], qkv_ps[:, 5 * HD : NQKV])
                sq = wk.tile([P, 5 * HD], F32, name=f"sq{i}", tag="sq")
                nc.gpsimd.tensor_mul(sq, qkv_sb[:, i, :], qkv_sb[:, i, :])
                nc.vector.reduce_sum(
                    ss_all[:, 5 * i : 5 * i + 5],
                    sq.rearrange("p (h d) -> p h d", d=HD),
                    axis=AXX,
                )

            # --- rsqrt via bitcast magic seed + 2 Newton steps (DVE only) ---
            ssc = ss_all[:, 20 * jb : 20 * jb + 20]
            rc = r_all[:, 20 * jb : 20 * jb + 20]
            mm = wk.tile([P, 20], F32, name=f"m{jb}", tag="m")
            # m = ss/HD + eps  (tensor_scalar: mult then add)
            nc.vector.tensor_scalar(
                out=mm, in0=ssc, scalar1=1.0 / HD, scalar2=RMS_EPS,
                op0=ALU.mult, op1=ALU.add,
            )
            tt = wk.tile([P, 20], F32, name=f"t{jb}", tag="t")
            nc.vector.tensor_scalar(
                out=tt.bitcast(I32), in0=mm.bitcast(I32),
                scalar1=1, scalar2=-1,
                op0=ALU.logical_shift_right, op1=ALU.bitwise_xor,
            )
            nc.vector.tensor_scalar(
                out=rc.bitcast(I32), in0=tt.bitcast(I32),
                scalar1=0x5F3759E0, scalar2=None, op0=ALU.add,
            )
            for _ in range(3):
                nc.vector.tensor_mul(tt, rc, rc)
                nc.vector.tensor_mul(tt, tt, mm)
                nc.vector.tensor_scalar(
                    out=tt, in0=tt, scalar1=-0.5, scalar2=1.5,
                    op0=ALU.mult, op1=ALU.add,
                )
                nc.vector.tensor_mul(rc, rc, tt)
            # fold gain/8 into the q scales (k slab untouched)
            rcv = rc.rearrange("p (t h) -> p t h", h=5)
            nc.vector.tensor_mul(
                rcv[:, :, 0:NH], rcv[:, :, 0:NH],
                qg8_sb[:, None, :].broadcast_to([P, 4, NH]),
            )

            # --- RoPE on the whole chunk ---
            qc = qkv_sb[:, 4 * jb : 4 * jb + 4, :]
            qcv = qc.rearrange("p t (h d) -> p (t h) d", d=HD)
            qks = rwk.tile([P, 4, 5 * HD], F32, name=f"qks{jb}", tag="qks")
            qksv = qks.rearrange("p t (h d) -> p (t h) d", d=HD)
            nc.vector.tensor_mul(
                qksv, qcv, rcv.rearrange("p t h -> p (t h)")[:, :, None].broadcast_to([P, 20, HD])
            )
            rot = rwk.tile([P, 4, 5 * HD], F32, name=f"rot{jb}", tag="rot")
            nc.vector.tensor_mul(
                rot.rearrange("p t f -> p (t f)"),
                qks.rearrange("p t f -> p (t f)"),
                cos_sb[:, 4 * jb : 4 * jb + 4, :].rearrange("p t f -> p (t f)"),
            )
            rotv = rot.rearrange("p t (h d) -> p (t h) d", d=HD)
            sinv = sin_sb[:, 4 * jb : 4 * jb + 4, :].rearrange(
                "p t (h d) -> p (t h) d", d=32
            )
            m2a = rwk.tile([P, 20, 32], F32, name=f"m2a{jb}", tag="m2a")
            nc.gpsimd.tensor_mul(m2a, qksv[:, :, 32:HD], sinv)
            m2b = rwk.tile([P, 20, 32], F32, name=f"m2b{jb}", tag="m2b")
            nc.gpsimd.tensor_mul(m2b, qksv[:, :, 0:32], sinv)
            nc.vector.tensor_add(rotv[:, :, 0:32], rotv[:, :, 0:32], m2a)
            nc.vector.tensor_sub(rotv[:, :, 32:HD], rotv[:, :, 32:HD], m2b)

            # --- transposes: 2 head-pair chunks + k, per s-tile ---
            for il in range(4):
                i = 4 * jb + il
                for pair in range(2):
                    trp = psmisc.tile([P, P], F32, name=f"tr{i}_{pair}", tag="mi")
                    nc.tensor.transpose(trp, rot[:, il, ts(pair, P)], ident)
                    nc.vector.tensor_copy(qT2[:, pair, ts(i, P)], trp)
                trk = psmisc.tile([HD, P], F32, name=f"trk{i}", tag="mi")
                nc.tensor.transpose(trk, rot[:, il, 4 * HD : 5 * HD], ident)
                nc.vector.tensor_copy(kTe[0:HD, ts(i, P)], trk)
                nc.scalar.copy(kTo[HD:P, ts(i, P)], trk)

        def b_block(j):
            """Attention for q-block j (both head pairs)."""
            nt = 4 * (j + 1)
            for pair in range(2):
                y0 = psy.tile([P, JW], F32, name=f"y{j}_{pair}_0", tag="y")
                y1 = psy.tile([P, JW], F32, name=f"y{j}_{pair}_1", tag="y")
                yy = (y0, y1)
                prev = None
                for t in range(nt):
                    m = t - 4 * j
                    w = JW if m < 0 else JW - P * m
                    ws = max(w, 256)        # keep score N >= 256 for fp32r rate
                    c0, c0s = JW - w, JW - ws
                    st = psst.tile([P, 2, JW], F32, name=f"st{j}_{pair}_{t}", tag="st")
                    p_sb = pwk.tile([P, 2, JW], BF16, name=f"p{j}_{pair}_{t}", tag="p")
                    qe = qT2[:, pair, ts(j, JW)]
                    nc.tensor.matmul(
                        st[:, 0, c0s:JW], lhsT=kTe[:, ts(t, P)], rhs=qe[:, c0s:JW],
                        start=True, stop=True,
                    )
                    nc.tensor.matmul(
                        st[:, 1, c0s:JW], lhsT=kTo[:, ts(t, P)], rhs=qe[:, c0s:JW],
                        start=True, stop=True,
                    )
                    # PV for the previous tile goes after this tile's scores so
                    # the PE stays one tile ahead of the exp.
                    if prev is not None:
                        pt, pw_, pc0 = prev
                        nc.tensor.matmul(
                            yy[0][:, pc0:JW], lhsT=v_sb[:, pt, :], rhs=pw_[:, 0, pc0:JW],
                            start=(pt == 0), stop=(pt == nt - 1),
                        )
                        nc.tensor.matmul(
                            yy[1][:, pc0:JW], lhsT=v_sb[:, pt, :], rhs=pw_[:, 1, pc0:JW],
                            start=(pt == 0), stop=(pt == nt - 1),
                        )
                    nc.scalar.activation(p_sb[:, :, c0:JW], st[:, :, c0:JW], ACT.Exp)
                    if m >= 0:
                        nc.vector.tensor_mul(
                            p_sb[:, :, c0:JW], p_sb[:, :, c0:JW],
                            tri_sb[:, None, 0:w].broadcast_to([P, 2, w]),
                        )
                    prev = (t, p_sb, c0)
                pt, pw_, pc0 = prev
                nc.tensor.matmul(
                    yy[0][:, pc0:JW], lhsT=v_sb[:, pt, :], rhs=pw_[:, 0, pc0:JW],
                    start=(pt == 0), stop=(pt == nt - 1),
                )
                nc.tensor.matmul(
                    yy[1][:, pc0:JW], lhsT=v_sb[:, pt, :], rhs=pw_[:, 1, pc0:JW],
                    start=(pt == 0), stop=(pt == nt - 1),
                )
                # softmax normalization: row 0 holds the denominator, the y
                # dims sit at rows 64-127 (aligned). Odd heads multiply
                # straight into y_sb rows 64-127; even heads stage + DMA-shift.
                for h in range(2):
                    rcp = nwk.tile([1, JW], F32, name=f"rc{j}_{pair}_{h}", tag="rcp")
                    nc.vector.reciprocal_approx_fast(rcp, yy[h][0:1, :])
                    bc = nwk.tile([P, JW], F32, name=f"bc{j}_{pair}_{h}", tag="bc")
                    nc.gpsimd.partition_broadcast(bc, rcp)
                    if h == 1:
                        nc.vector.tensor_mul(
                            y_sb[HD:P, pair, ts(j, JW)], yy[h][HD:P, :], bc[HD:P, :]
                        )
                    else:
                        yt = nwk.tile([P, JW], BF16, name=f"yt{j}_{pair}", tag="yt")
                        nc.vector.tensor_mul(yt[HD:P, :], yy[h][HD:P, :], bc[HD:P, :])
                        nc.sync.dma_start(
                            out=y_sb[0:HD, pair, ts(j, JW)], in_=yt[HD:P, :]
                        )

        def c_block(j):
            """Partial output projection for q-block j."""
            for mtile in range(D // P):
                op = psmisc.tile([P, JW], F32, name=f"op{j}_{mtile}", tag="mi")
                for c in range(2):
                    nc.tensor.matmul(
                        op,
                        lhsT=wp_sb[:, c, ts(mtile, P)],
                        rhs=y_sb[:, c, ts(j, JW)],
                        start=(c == 0),
                        stop=(c == 1),
                    )
                o_sb = nwk.tile([P, JW], BF16, name=f"o{j}_{mtile}", tag="o")
                if mtile % 2 == 0:
                    nc.vector.tensor_copy(o_sb, op)
                else:
                    nc.scalar.copy(o_sb, op)
                nc.sync.dma_start(out=ypt[ts(mtile, P), ts(j, JW)], in_=o_sb)

        u_chunk(0)
        u_chunk(1)
        b_block(0)
        u_chunk(2)
        b_block(1)
        c_block(0)
        u_chunk(3)
        b_block(2)
        c_block(1)
        b_block(3)
        c_block(2)
        c_block(3)


_PROG = None


def _get_program():
    global _PROG
    if _PROG is None:
        _PROG = _build_program()
    return _PROG


def _bf16(a):
    import ml_dtypes

    return np.ascontiguousarray(a.astype(ml_dtypes.bfloat16))


def _host_tables():
    inv_freq = (
        1.0 / (ROPE_BASE ** (np.arange(0, HD, 2, dtype=np.float32) / HD))
    ).astype(np.float32)
    t = np.arange(S, dtype=np.float32)
    freqs = t[:, None] * inv_freq[None, :]  # [S, 32]
    cosf = np.cos(freqs).astype(np.float32)
    sinf = np.sin(freqs).astype(np.float32)
    cosd = np.concatenate([cosf, cosf], axis=1)  # [S, 64]
    # duplicate per slab (4 q heads + k share tables): [S, 5, 64]
    cos5 = np.repeat(cosd[:, None, :], 5, axis=1)
    sin5 = np.repeat(sinf[:, None, :], 5, axis=1)
    cos5 = np.ascontiguousarray(
        cos5.reshape(NST, P, 5 * HD).transpose(1, 0, 2).reshape(P, NST * 5 * HD)
    )
    sin5 = np.ascontiguousarray(
        sin5.reshape(NST, P, 5 * 32).transpose(1, 0, 2).reshape(P, NST * 5 * 32)
    )
    p_idx = np.arange(P)[:, None]
    x_idx = np.arange(JW)[None, :]
    tri = _bf16((x_idx >= p_idx).astype(np.float32))  # [128, 512]
    return cos5, sin5, tri


def _in_maps(x, Wq, Wk, Wv, Wproj, q_gain):
    cos5, sin5, tri = _host_tables()
    maps = []
    for core in range(NC):
        b, g = divmod(core, KV)
        xT = np.ascontiguousarray(x[b].T)  # [D, S]
        wqkv = np.ascontiguousarray(
            np.concatenate(
                [
                    Wq[g * GD : (g + 1) * GD].T,
                    Wk[g * HD : (g + 1) * HD].T,
                    Wv[g * HD : (g + 1) * HD].T,
                ],
                axis=1,
            )
        )  # [D, 384]
        wsl = Wproj[:, g * GD : (g + 1) * GD].T.reshape(NH, HD, D)  # [head, d, m]
        wp2 = _bf16(
            np.stack(
                [
                    np.concatenate([wsl[0], wsl[1]], axis=0),
                    np.concatenate([wsl[2], wsl[3]], axis=0),
                ],
                axis=1,
            ).reshape(P, 2 * D)
        )
        qg8 = np.ascontiguousarray(
            (q_gain[g * NH : (g + 1) * NH] / 8.0).astype(np.float32).reshape(1, NH)
        )
        maps.append(
            {
                "xT": xT,
                "wqkv": wqkv,
                "wp2": wp2,
                "cos5": cos5,
                "sin5": sin5,
                "tri": tri,
                "qg8": qg8,
            }
        )
    return maps


def kernel(x, Wq, Wk, Wv, Wproj, q_gain, _collect=None):
    x = np.asarray(x, dtype=np.float32)
    Wq = np.asarray(Wq, dtype=np.float32)
    Wk = np.asarray(Wk, dtype=np.float32)
    Wv = np.asarray(Wv, dtype=np.float32)
    Wproj = np.asarray(Wproj, dtype=np.float32)
    q_gain = np.asarray(q_gain, dtype=np.float32)

    nc = _get_program()
    maps = _in_maps(x, Wq, Wk, Wv, Wproj, q_gain)
    res = run_bass_kernel_spmd(nc, maps, core_ids=list(range(NC)))
    if _collect is not None:
        _collect.append(res)

    out = np.zeros((B, S, D), dtype=np.float32)
    for core in range(NC):
        b, _ = divmod(core, KV)
        out[b] += res.results[core]["ypt"].astype(np.float32).T
    return out.astype(np.float32)
